# revision 10
# baseline (speedup 1.0000x reference)
"""Group-limited MoE router kernel for Trainium2 (Bass/Tile), 8-core SPMD.

Per token (row of 256 experts):
  scores = sigmoid(logits); biased = scores + bias
  group_score[g] = top2sum(biased[g*32:(g+1)*32]) for 8 groups
  keep top-4 groups, mask the rest; topk_ids = top-8 of masked biased
  weights = scores[topk_ids] renormalized to sum 1, * 2.5

Strategy (v2): quantize biased to a positive int grid (2^-14 cells) and
pack an 8-bit score approximation into the low byte:
  ival   = int(scores*2^14 + bias*2^14 + 2^15)        in [22938, 58168]
  packed = ival*256 + scores*253                      exact int < 2^24 in fp32
Group top-2 via grouped-max / match_replace / grouped-max on ival.
Final top-8 via max8 + find_index8 on masked packed: positions give the
expert ids exactly; low 8 bits of the values give the scores for the
renormalized weights (score scale cancels in the renorm).

Engine split: ScalarE does sigmoid/score-scale/hi-extract; GpSimd does the
three elementwise passes (ival, packed, mask-apply); VectorE does the
reduces, match_replace, max8/find_index8 and small glue.

Data-parallel over tokens: 131072 -> 8 cores x 16384; 128 tokens per
partition-slab, S=4 slabs batched per instruction block.
"""

import numpy as np

TOKENS = 131072
E = 256
G = 8
EPG = 32
K = 8
N_CORES = 8

P = 128
S = 4  # slabs per instruction block

IV_SCALE = 16384.0  # 2^14 quantization of biased
IV_OFF = 32768.0  # keep ival strictly positive
MAGIC = 12582912.0  # 3*2^22: float add forces round-to-int in [2^23, 2^24)
NEGP = -131072.0  # -2^17 group mask in packed (ival + score) domain
MATCH_IMM = -1.0  # replaces group maxima (all ival > 0)
HI_SCALE = 1.0 / 256.0
HI_BIAS = -0.496  # centers score-0.496 in (-0.5, 0.5) for round-nearest
WSUM_PRE = 1.0 / 2.5  # w = sq / (0.4 * sum(sq)) == 2.5 * score / sum(score)


def build_kernel(tpc: int):
    import concourse.bass as bass
    import concourse.bacc as bacc
    import concourse.mybir as mybir
    from concourse.tile import TileContext

    f32 = mybir.dt.float32
    i32 = mybir.dt.int32
    u32 = mybir.dt.uint32
    Alu = mybir.AluOpType
    Act = mybir.ActivationFunctionType
    X = mybir.AxisListType.X

    nc = bacc.Bacc()
    logits_d = nc.declare_dram_parameter("logits", [tpc, E], f32, isOutput=False)
    # host precomputes biasq = bias*2^14 + 2^15 + 3*2^22 (magic rounder)
    biasq_d = nc.declare_dram_parameter("biasq", [1, E], f32, isOutput=False)
    w_d = nc.declare_dram_parameter("weights", [tpc, K], f32, isOutput=True)
    i_d = nc.declare_dram_parameter("ids", [tpc, K], u32, isOutput=True)

    assert tpc % (P * S) == 0
    n_blk = tpc // (P * S)
    SE = S * E
    SG = S * G
    SK = S * K

    with TileContext(nc) as tc:
        with (
            tc.tile_pool(name="const", bufs=1) as cpool,
            tc.tile_pool(name="big", bufs=4) as big,
            tc.tile_pool(name="sm", bufs=3) as sm,
            tc.tile_pool(name="out", bufs=3) as outp,
        ):
            biasq = cpool.tile([P, E], f32)
            nc.gpsimd.dma_start(out=biasq, in_=biasq_d[:].to_broadcast([P, E]))
            # pre-touch so consumers don't each wait on the DMA
            dummy = cpool.tile([P, 1], f32)
            nc.vector.tensor_copy(out=dummy, in_=biasq[:, 0:1])

            for b in range(n_blk):
                t0 = b * P * S
                x = big.tile([P, SE], f32, tag="x")
                nc.sync.dma_start(
                    out=x.rearrange("p (s e) -> p s e", e=E),
                    in_=logits_d[t0 : t0 + S * P, :].rearrange(
                        "(s p) e -> p s e", p=P
                    ),
                )

                scores = big.tile([P, SE], f32, tag="scores")
                nc.scalar.activation(out=scores, in_=x, func=Act.Sigmoid)
                s2k = big.tile([P, SE], f32, tag="s2k")
                nc.scalar.activation(out=s2k, in_=scores, func=Act.Copy, scale=IV_SCALE)

                # ivalm = round(scores*2^14 + bias*2^14 + 2^15) + 3*2^22
                # (magic-number rounding: result lands in [2^23, 2^24) where
                #  fp32 ulp is 1, so the add itself quantizes)  [GpSimd]
                ivalm = big.tile([P, SE], f32, tag="ivalm")
                nc.gpsimd.tensor_tensor(
                    out=ivalm,
                    in0=s2k,
                    in1=biasq.unsqueeze(1).to_broadcast([P, S, E]),
                    op=Alu.add,
                )
                # iv1 = ivalm - magic = quantized ival (exact subtract)  [ScalarE]
                iv1 = big.tile([P, SE], f32, tag="iv1")
                nc.scalar.activation(
                    out=iv1, in_=ivalm, func=Act.Copy, bias=-MAGIC
                )
                # packed = ival + score: score lives in the fraction bits
                # (fp32 ulp <= 2^-8 for ival < 2^16 keeps ~8 score bits)
                packed = big.tile([P, SE], f32, tag="packed")
                nc.gpsimd.tensor_tensor(out=packed, in0=iv1, in1=scores, op=Alu.add)

                # group top-2 on ivalm [VectorE]
                m1 = sm.tile([P, SG], f32, tag="m1")
                nc.vector.tensor_reduce(
                    out=m1,
                    in_=ivalm.rearrange("p (sg e) -> p sg e", e=EPG),
                    axis=X,
                    op=Alu.max,
                )
                rep = big.tile([P, SE], f32, tag="rep")
                for s in range(S):
                    nc.vector.match_replace(
                        out=rep[:, s * E : (s + 1) * E],
                        in_to_replace=m1[:, s * G : (s + 1) * G],
                        in_values=ivalm[:, s * E : (s + 1) * E],
                        imm_value=MATCH_IMM,
                    )
                m2 = sm.tile([P, SG], f32, tag="m2")
                nc.vector.tensor_reduce(
                    out=m2,
                    in_=rep.rearrange("p (sg e) -> p sg e", e=EPG),
                    axis=X,
                    op=Alu.max,
                )
                gs = sm.tile([P, SG], f32, tag="gs")
                nc.vector.tensor_tensor(out=gs, in0=m1, in1=m2, op=Alu.add)

                # top-4 groups via rank count: drop g if #{j: gs_j >= gs_g} > 4
                cmp = sm.tile([P, SG * G], f32, tag="cmp")
                gs3 = gs.rearrange("p (s g) -> p s g", g=G)
                nc.vector.tensor_tensor(
                    out=cmp.rearrange("p (s i j) -> p s i j", i=G, j=G),
                    in0=gs3.unsqueeze(3).to_broadcast([P, S, G, G]),
                    in1=gs3.unsqueeze(2).to_broadcast([P, S, G, G]),
                    op=Alu.is_le,
                )
                cnt = sm.tile([P, SG], f32, tag="cnt")
                nc.vector.tensor_reduce(
                    out=cnt,
                    in_=cmp.rearrange("p (sg j) -> p sg j", j=G),
                    axis=X,
                    op=Alu.add,
                )
                negp = sm.tile([P, SG], f32, tag="negp")
                nc.vector.tensor_scalar(
                    out=negp,
                    in0=cnt,
                    scalar1=4.5,
                    scalar2=NEGP,
                    op0=Alu.is_gt,
                    op1=Alu.mult,
                )

                maskedP = big.tile([P, SE], f32, tag="maskedP")
                nc.gpsimd.tensor_tensor(
                    out=maskedP,
                    in0=packed,
                    in1=negp.unsqueeze(2).to_broadcast([P, SG, EPG]),
                    op=Alu.add,
                )

                # final top-8: values (scores in low byte) + positions (ids)
                p8 = sm.tile([P, SK], f32, tag="p8")
                idx8 = outp.tile([P, SK], u32, tag="idx8")
                for s in range(S):
                    sl = slice(s * K, (s + 1) * K)
                    nc.vector.max(out=p8[:, sl], in_=maskedP[:, s * E : (s + 1) * E])
                    nc.vector.max_index(
                        out=idx8[:, sl],
                        in_max=p8[:, sl],
                        in_values=maskedP[:, s * E : (s + 1) * E],
                    )

                # sq = p8 - round(p8 - 0.496); weights = sq/(0.4*sum(sq))
                # round() via magic-number add (mode-independent, RNE fp add)
                him = sm.tile([P, SK], f32, tag="him")
                nc.scalar.activation(
                    out=him, in_=p8, func=Act.Copy, bias=HI_BIAS
                )
                hif = sm.tile([P, SK], f32, tag="hif")
                nc.vector.tensor_scalar(
                    out=hif,
                    in0=him,
                    scalar1=MAGIC,
                    scalar2=-MAGIC,
                    op0=Alu.add,
                    op1=Alu.add,
                )
                sqv = sm.tile([P, SK], f32, tag="sqv")
                nc.vector.scalar_tensor_tensor(
                    out=sqv,
                    in0=hif,
                    scalar=-1.0,
                    in1=p8,
                    op0=Alu.mult,
                    op1=Alu.add,
                )
                wsum = sm.tile([P, S], f32, tag="wsum")
                nc.vector.tensor_reduce(
                    out=wsum,
                    in_=sqv.rearrange("p (s k) -> p s k", k=K),
                    axis=X,
                    op=Alu.add,
                )
                nc.scalar.activation(
                    out=wsum, in_=wsum, func=Act.Copy, scale=WSUM_PRE
                )
                rcp = sm.tile([P, S], f32, tag="rcp")
                nc.vector.reciprocal(out=rcp, in_=wsum)

                wout = outp.tile([P, SK], f32, tag="wout")
                nc.vector.tensor_tensor(
                    out=wout.rearrange("p (s k) -> p s k", k=K),
                    in0=sqv.rearrange("p (s k) -> p s k", k=K),
                    in1=rcp.unsqueeze(2).to_broadcast([P, S, K]),
                    op=Alu.mult,
                )

                rows = slice(t0, t0 + S * P)
                nc.sync.dma_start(
                    out=w_d[rows, :].rearrange("(s p) k -> p s k", p=P),
                    in_=wout.rearrange("p (s k) -> p s k", k=K),
                )
                nc.sync.dma_start(
                    out=i_d[rows, :].rearrange("(s p) k -> p s k", p=P),
                    in_=idx8.rearrange("p (s k) -> p s k", k=K),
                )

    nc.finalize()
    return nc


_NC_CACHE = {}


def _get_nc(tpc: int):
    if tpc not in _NC_CACHE:
        _NC_CACHE[tpc] = build_kernel(tpc)
    return _NC_CACHE[tpc]


def kernel(router_logits: np.ndarray, expert_bias: np.ndarray, _trace: bool = False):
    from concourse.bass_utils import run_bass_kernel_spmd

    router_logits = np.asarray(router_logits, dtype=np.float32)
    expert_bias = np.asarray(expert_bias, dtype=np.float32)
    tokens = router_logits.shape[0]
    assert tokens % N_CORES == 0
    tpc = tokens // N_CORES

    nc = _get_nc(tpc)
    biasq = (expert_bias.astype(np.float64) * IV_SCALE + IV_OFF + MAGIC).astype(
        np.float32
    ).reshape(1, E)
    in_maps = [
        {
            "logits": np.ascontiguousarray(router_logits[c * tpc : (c + 1) * tpc]),
            "biasq": biasq,
        }
        for c in range(N_CORES)
    ]
    res = run_bass_kernel_spmd(
        nc, in_maps, core_ids=list(range(N_CORES)), trace=_trace
    )
    weights = np.concatenate([r["weights"] for r in res.results], axis=0)
    ids = np.concatenate([r["ids"] for r in res.results], axis=0).astype(np.int32)
    if _trace:
        kernel.last_exec_time_ns = res.exec_time_ns
        kernel.last_mean_exec_time_ns = res.mean_exec_time_ns
    return weights, ids


# revision 12
# speedup vs baseline: 1.0644x; 1.0644x over previous
"""Group-limited MoE router kernel for Trainium2 (Bass/Tile), 8-core SPMD.

Per token (row of 256 experts):
  scores = sigmoid(logits); biased = scores + bias
  group_score[g] = top2sum(biased[g*32:(g+1)*32]) for 8 groups
  keep top-4 groups, mask the rest; topk_ids = top-8 of masked biased
  weights = scores[topk_ids] renormalized to sum 1, * 2.5

Strategy (v2): quantize biased to a positive int grid (2^-14 cells) and
pack an 8-bit score approximation into the low byte:
  ival   = int(scores*2^14 + bias*2^14 + 2^15)        in [22938, 58168]
  packed = ival*256 + scores*253                      exact int < 2^24 in fp32
Group top-2 via grouped-max / match_replace / grouped-max on ival.
Final top-8 via max8 + find_index8 on masked packed: positions give the
expert ids exactly; low 8 bits of the values give the scores for the
renormalized weights (score scale cancels in the renorm).

Engine split: ScalarE does sigmoid/score-scale/hi-extract; GpSimd does the
three elementwise passes (ival, packed, mask-apply); VectorE does the
reduces, match_replace, max8/find_index8 and small glue.

Data-parallel over tokens: 131072 -> 8 cores x 16384; 128 tokens per
partition-slab, S=4 slabs batched per instruction block.
"""

import numpy as np

TOKENS = 131072
E = 256
G = 8
EPG = 32
K = 8
N_CORES = 8

P = 128
S = 8  # slabs per instruction block

IV_SCALE = 16384.0  # 2^14 quantization of biased
IV_OFF = 32768.0  # keep ival strictly positive
MAGIC = 12582912.0  # 3*2^22: float add forces round-to-int in [2^23, 2^24)
NEGP = -131072.0  # -2^17 group mask in packed (ival + score) domain
MATCH_IMM = -1.0  # replaces group maxima (all ival > 0)
HI_SCALE = 1.0 / 256.0
HI_BIAS = -0.496  # centers score-0.496 in (-0.5, 0.5) for round-nearest
WSUM_PRE = 1.0 / 2.5  # w = sq / (0.4 * sum(sq)) == 2.5 * score / sum(score)


def build_kernel(tpc: int):
    import concourse.bass as bass
    import concourse.bacc as bacc
    import concourse.mybir as mybir
    from concourse.tile import TileContext

    f32 = mybir.dt.float32
    i32 = mybir.dt.int32
    u32 = mybir.dt.uint32
    Alu = mybir.AluOpType
    Act = mybir.ActivationFunctionType
    X = mybir.AxisListType.X

    nc = bacc.Bacc()
    logits_d = nc.declare_dram_parameter("logits", [tpc, E], f32, isOutput=False)
    # host precomputes biasq = bias*2^14 + 2^15 + 3*2^22 (magic rounder)
    biasq_d = nc.declare_dram_parameter("biasq", [1, E], f32, isOutput=False)
    w_d = nc.declare_dram_parameter("weights", [tpc, K], f32, isOutput=True)
    i_d = nc.declare_dram_parameter("ids", [tpc, K], u32, isOutput=True)

    assert tpc % (P * S) == 0
    n_blk = tpc // (P * S)
    SE = S * E
    SG = S * G
    SK = S * K

    with TileContext(nc) as tc:
        with (
            tc.tile_pool(name="const", bufs=1) as cpool,
            tc.tile_pool(name="big", bufs=2) as big,
            tc.tile_pool(name="sm", bufs=3) as sm,
            tc.tile_pool(name="out", bufs=3) as outp,
        ):
            biasq = cpool.tile([P, E], f32)
            nc.gpsimd.dma_start(out=biasq, in_=biasq_d[:].to_broadcast([P, E]))
            # pre-touch so consumers don't each wait on the DMA
            dummy = cpool.tile([P, 1], f32)
            nc.vector.tensor_copy(out=dummy, in_=biasq[:, 0:1])

            for b in range(n_blk):
                t0 = b * P * S
                x = big.tile([P, SE], f32, tag="x")
                nc.sync.dma_start(
                    out=x.rearrange("p (s e) -> p s e", e=E),
                    in_=logits_d[t0 : t0 + S * P, :].rearrange(
                        "(s p) e -> p s e", p=P
                    ),
                )

                scores = big.tile([P, SE], f32, tag="scores")
                nc.scalar.activation(out=scores, in_=x, func=Act.Sigmoid)
                s2k = big.tile([P, SE], f32, tag="s2k")
                nc.scalar.activation(out=s2k, in_=scores, func=Act.Copy, scale=IV_SCALE)

                # ivalm = round(scores*2^14 + bias*2^14 + 2^15) + 3*2^22
                # (magic-number rounding: result lands in [2^23, 2^24) where
                #  fp32 ulp is 1, so the add itself quantizes)  [GpSimd]
                ivalm = big.tile([P, SE], f32, tag="ivalm")
                nc.gpsimd.tensor_tensor(
                    out=ivalm,
                    in0=s2k,
                    in1=biasq.unsqueeze(1).to_broadcast([P, S, E]),
                    op=Alu.add,
                )
                # iv1 = ivalm - magic = quantized ival (exact subtract)  [ScalarE]
                iv1 = big.tile([P, SE], f32, tag="iv1")
                nc.scalar.activation(
                    out=iv1, in_=ivalm, func=Act.Copy, bias=-MAGIC
                )
                # packed = ival + score: score lives in the fraction bits
                # (fp32 ulp <= 2^-8 for ival < 2^16 keeps ~8 score bits)
                packed = big.tile([P, SE], f32, tag="packed")
                nc.gpsimd.tensor_tensor(out=packed, in0=iv1, in1=scores, op=Alu.add)

                # group top-2 on ivalm [VectorE]
                m1 = sm.tile([P, SG], f32, tag="m1")
                nc.vector.tensor_reduce(
                    out=m1,
                    in_=ivalm.rearrange("p (sg e) -> p sg e", e=EPG),
                    axis=X,
                    op=Alu.max,
                )
                rep = big.tile([P, SE], f32, tag="rep")
                for s in range(S):
                    nc.vector.match_replace(
                        out=rep[:, s * E : (s + 1) * E],
                        in_to_replace=m1[:, s * G : (s + 1) * G],
                        in_values=ivalm[:, s * E : (s + 1) * E],
                        imm_value=MATCH_IMM,
                    )
                m2 = sm.tile([P, SG], f32, tag="m2")
                nc.vector.tensor_reduce(
                    out=m2,
                    in_=rep.rearrange("p (sg e) -> p sg e", e=EPG),
                    axis=X,
                    op=Alu.max,
                )
                gs = sm.tile([P, SG], f32, tag="gs")
                nc.gpsimd.tensor_tensor(out=gs, in0=m1, in1=m2, op=Alu.add)

                # top-4 groups via rank count: drop g if #{j: gs_j >= gs_g} > 4
                cmp = sm.tile([P, SG * G], f32, tag="cmp")
                gs3 = gs.rearrange("p (s g) -> p s g", g=G)
                nc.vector.tensor_tensor(
                    out=cmp.rearrange("p (s i j) -> p s i j", i=G, j=G),
                    in0=gs3.unsqueeze(3).to_broadcast([P, S, G, G]),
                    in1=gs3.unsqueeze(2).to_broadcast([P, S, G, G]),
                    op=Alu.is_le,
                )
                cnt = sm.tile([P, SG], f32, tag="cnt")
                nc.vector.tensor_reduce(
                    out=cnt,
                    in_=cmp.rearrange("p (sg j) -> p sg j", j=G),
                    axis=X,
                    op=Alu.add,
                )
                negp = sm.tile([P, SG], f32, tag="negp")
                nc.vector.tensor_scalar(
                    out=negp,
                    in0=cnt,
                    scalar1=4.5,
                    scalar2=NEGP,
                    op0=Alu.is_gt,
                    op1=Alu.mult,
                )

                maskedP = big.tile([P, SE], f32, tag="maskedP")
                nc.gpsimd.tensor_tensor(
                    out=maskedP,
                    in0=packed,
                    in1=negp.unsqueeze(2).to_broadcast([P, SG, EPG]),
                    op=Alu.add,
                )

                # final top-8: values (scores in low byte) + positions (ids)
                p8 = sm.tile([P, SK], f32, tag="p8")
                idx8 = outp.tile([P, SK], u32, tag="idx8")
                for s in range(S):
                    sl = slice(s * K, (s + 1) * K)
                    nc.vector.max(out=p8[:, sl], in_=maskedP[:, s * E : (s + 1) * E])
                    nc.vector.max_index(
                        out=idx8[:, sl],
                        in_max=p8[:, sl],
                        in_values=maskedP[:, s * E : (s + 1) * E],
                    )

                # sq = p8 - round(p8 - 0.496); weights = sq/(0.4*sum(sq))
                # round() via magic-number add (mode-independent, RNE fp add)
                him = sm.tile([P, SK], f32, tag="him")
                nc.scalar.activation(
                    out=him, in_=p8, func=Act.Copy, bias=HI_BIAS
                )
                hif = sm.tile([P, SK], f32, tag="hif")
                nc.vector.tensor_scalar(
                    out=hif,
                    in0=him,
                    scalar1=MAGIC,
                    scalar2=-MAGIC,
                    op0=Alu.add,
                    op1=Alu.add,
                )
                sqv = sm.tile([P, SK], f32, tag="sqv")
                nc.gpsimd.tensor_tensor(
                    out=sqv, in0=p8, in1=hif, op=Alu.subtract
                )
                wsum = sm.tile([P, S], f32, tag="wsum")
                nc.vector.tensor_reduce(
                    out=wsum,
                    in_=sqv.rearrange("p (s k) -> p s k", k=K),
                    axis=X,
                    op=Alu.add,
                )
                nc.scalar.activation(
                    out=wsum, in_=wsum, func=Act.Copy, scale=WSUM_PRE
                )
                rcp = sm.tile([P, S], f32, tag="rcp")
                nc.vector.reciprocal(out=rcp, in_=wsum)

                wout = outp.tile([P, SK], f32, tag="wout")
                nc.gpsimd.tensor_tensor(
                    out=wout.rearrange("p (s k) -> p s k", k=K),
                    in0=sqv.rearrange("p (s k) -> p s k", k=K),
                    in1=rcp.unsqueeze(2).to_broadcast([P, S, K]),
                    op=Alu.mult,
                )

                rows = slice(t0, t0 + S * P)
                nc.sync.dma_start(
                    out=w_d[rows, :].rearrange("(s p) k -> p s k", p=P),
                    in_=wout.rearrange("p (s k) -> p s k", k=K),
                )
                nc.sync.dma_start(
                    out=i_d[rows, :].rearrange("(s p) k -> p s k", p=P),
                    in_=idx8.rearrange("p (s k) -> p s k", k=K),
                )

    nc.finalize()
    return nc


_NC_CACHE = {}


def _get_nc(tpc: int):
    if tpc not in _NC_CACHE:
        _NC_CACHE[tpc] = build_kernel(tpc)
    return _NC_CACHE[tpc]


def kernel(router_logits: np.ndarray, expert_bias: np.ndarray, _trace: bool = False):
    from concourse.bass_utils import run_bass_kernel_spmd

    router_logits = np.asarray(router_logits, dtype=np.float32)
    expert_bias = np.asarray(expert_bias, dtype=np.float32)
    tokens = router_logits.shape[0]
    assert tokens % N_CORES == 0
    tpc = tokens // N_CORES

    nc = _get_nc(tpc)
    biasq = (expert_bias.astype(np.float64) * IV_SCALE + IV_OFF + MAGIC).astype(
        np.float32
    ).reshape(1, E)
    in_maps = [
        {
            "logits": np.ascontiguousarray(router_logits[c * tpc : (c + 1) * tpc]),
            "biasq": biasq,
        }
        for c in range(N_CORES)
    ]
    res = run_bass_kernel_spmd(
        nc, in_maps, core_ids=list(range(N_CORES)), trace=_trace
    )
    weights = np.concatenate([r["weights"] for r in res.results], axis=0)
    ids = np.concatenate([r["ids"] for r in res.results], axis=0).astype(np.int32)
    if _trace:
        kernel.last_exec_time_ns = res.exec_time_ns
        kernel.last_mean_exec_time_ns = res.mean_exec_time_ns
    return weights, ids
